# revision 22
# baseline (speedup 1.0000x reference)
"""Trainium2 Bass kernel for nn_ExpansionContrastModule.

Strategy: pure data parallel over 8 cores; each core processes half of one
batch image (128 of 256 rows), with a 3-row halo so the dilated contrast
convs and the 3x3 mas conv need no cross-core traffic.

v4 (HW-measured 201us/rep vs 246us for the f32 baseline):
- cen is read once as bf16 for the x-conv (per-group cg tiles, released
  right after the x-conv) and re-read bf16 for the final multiply via
  transient cf tiles, so consecutive halves pipeline instead of
  serializing on cen residency.
- output is written bf16 and converted to f32 on the host.
- prologue loads (cg/edges/mas) issue on the nc.sync HWDGE ring; tail
  DMAs (cf loads, out writes) on the nc.scalar ring, so the next rep's
  input loads never queue behind this rep's tail writes.
- mas input is column-padded to 258 so every mas9 row read is a full
  512B run (no <512B DMA penalty, no edge memsets).
- the two-shift min/mean/max combine is folded to a*min + b*max
  (min+max == sum for two elements) -> 2 bc matmul terms instead of 3.
- the gate's linear terms (s1*mm + s3) are folded into the PE broadcast
  matmul via an extra ones-row; only ct = om*(s2*mm+s0) needs the DVE.
- elementwise work stays on DVE (bf16 2x) with the sum trees included:
  offloading chains to Pool measured 3-5x worse than the cost model.
"""
import sys
import ml_dtypes
import numpy as np

sys.path.insert(0, "/opt/trn_rl_repo")

import concourse.bass as bass
import concourse.bacc as bacc
import concourse.mybir as mybir
from concourse.tile import TileContext
from concourse.bass_utils import run_bass_kernel_spmd

F32 = mybir.dt.float32
AF = mybir.ActivationFunctionType
ALU = mybir.AluOpType

N_CORES = 8
C = 128        # input channels
CR = 16        # reduced channels
H = W = 256
CH = 128       # rows per core (half an image)
MH = 2         # macro-halves per core
HB = 64        # rows per macro-half
G = 8          # row-groups per macro-half
GR = 8         # rows per group
XR = GR + 6    # x tile rows (3-row halo each side)
XP = 4         # x tile left/right col pad (4 for bf16 4B alignment)
XW = W + 2 * XP  # x tile cols
QF = GR * W    # free elems per slab (2048)
BF = mybir.dt.bfloat16

BN_EPS = 1e-5

# cbf column layout
CB_WIN = 0          # w_in_blk       [C, 8*C]
CB_BC = 1024        # bc2 (a,b)      [C, 2*C]
CB_WOUT = 1280      # wout_lhsT      [C, 8]
CB_MAS = 1288       # mas_lhsT       [72, 8]
CB_BCT = 1296       # bcast_ct       [8, 8*C]
CB_BMM = 2320       # bcast_mm       [9, 8*C]
CB_W = 3344

# cf32/scal column indices
S_W0, S_W1_4, S_W2, S_V0, S_V1_2, S_V2 = 0, 1, 2, 3, 4, 5
S_G0, S_G1, S_G2, S_BOUT, S_MB1, S_MW2, S_MB2 = 6, 7, 8, 9, 10, 11, 12

_CACHE = {}

# elementwise routing knobs (bisect HW vs cost-model behavior)
R_SUM_POOL = False    # sum tree on Pool engine instead of DVE
R_TSP_ACT = False     # q1/qa/qb scale ops on ACT instead of DVE
R_MSIL_POOL = False   # mas silu multiply on Pool
R_FM_POOL = True      # alternate final-mult chunks onto the idle Pool engine
R_XW_DVE = True       # interior x-row writes via DVE tensor_scalar_add


def _fl(t):
    """Flatten the two free dims of a [P, a, b] tile AP into [P, a*b]."""
    return t[:, :, :].rearrange("p a b -> p (a b)")


def build_nc(loop_reps=0):
    nc = bacc.Bacc("TRN2", target_bir_lowering=False, debug=False,
                   num_devices=N_CORES)
    cen_b = nc.dram_tensor("cen_bf", [C, CH + 6, W], BF, kind="ExternalInput")
    mas_p = nc.dram_tensor("mas", [CH + 8, W + 2], BF, kind="ExternalInput")
    cbf_d = nc.dram_tensor("cbf", [C, CB_W], BF, kind="ExternalInput")
    cf32_d = nc.dram_tensor("cf32", [C, 18], F32, kind="ExternalInput")
    out_p = nc.dram_tensor("out", [C, CH, W], BF, kind="ExternalOutput")

    with TileContext(nc) as tc:
        import contextlib
        _stk = contextlib.ExitStack()
        with _stk:
            cpool = _stk.enter_context(tc.tile_pool(name="const", bufs=1))
            cgpool = _stk.enter_context(tc.tile_pool(name="cg", bufs=9))
            cepool = _stk.enter_context(tc.tile_pool(name="ce", bufs=1))
            xpool = _stk.enter_context(tc.tile_pool(name="x", bufs=2))
            xopool = _stk.enter_context(tc.tile_pool(name="xo", bufs=2))
            dpool = _stk.enter_context(tc.tile_pool(name="d", bufs=3))
            opool = _stk.enter_context(tc.tile_pool(name="o", bufs=4))
            trpool = _stk.enter_context(tc.tile_pool(name="tr", bufs=5))
            tspool = _stk.enter_context(tc.tile_pool(name="tsb", bufs=2))
            typool = _stk.enter_context(tc.tile_pool(name="ty", bufs=1))
            zpool = _stk.enter_context(tc.tile_pool(name="z", bufs=1))
            qmpool = _stk.enter_context(tc.tile_pool(name="qm", bufs=2))
            qcpool = _stk.enter_context(tc.tile_pool(name="qc", bufs=2))
            mas9pool = _stk.enter_context(tc.tile_pool(name="m9", bufs=2))
            masspool = _stk.enter_context(tc.tile_pool(name="ms", bufs=4))
            gbpool = _stk.enter_context(tc.tile_pool(name="gb", bufs=2))
            cfpool = _stk.enter_context(tc.tile_pool(name="cf", bufs=2))
            outpool = _stk.enter_context(tc.tile_pool(name="ot", bufs=2))
            pspool = _stk.enter_context(tc.tile_pool(name="ps", bufs=2, space="PSUM"))
            pbpool = _stk.enter_context(tc.tile_pool(name="pb", bufs=2, space="PSUM"))
            pupool = _stk.enter_context(tc.tile_pool(name="pu", bufs=2, space="PSUM"))
            # ---- constants ----
            cbf_sb = cpool.tile([C, CB_W], BF, tag="c_bf")
            cf32_sb = cpool.tile([C, 18], F32, tag="c_f32")
            nc.sync.dma_start(out=cbf_sb[:], in_=cbf_d[:])
            nc.sync.dma_start(out=cf32_sb[:], in_=cf32_d[:])

            def sc(col, p=C):
                return cf32_sb[0:p, 5 + col:6 + col]

            def emit_half(mh):
                base = mh * HB

                # ---- load this half's cen rows as 8 per-group tiles
                # (cg[g] row r = image row r0+base+8g+r); kept resident: the
                # x-conv matmuls and the final multiply both read them, and
                # each tile releases right after its group's final multiply.
                cg = []
                for g in range(G):
                    cg_t = cgpool.tile([C, GR, W], BF, tag="cg")
                    nc.sync.dma_start(
                        out=cg_t[:],
                        in_=cen_b[:, base + 3 + 8 * g: base + 3 + 8 * g + 8, :])
                    cg.append(cg_t)
                et = cepool.tile([C, 3, W], BF, tag="et")
                nc.sync.dma_start(out=et[:], in_=cen_b[:, base:base + 3, :])
                eb = cepool.tile([C, 3, W], BF, tag="eb")
                nc.sync.dma_start(out=eb[:],
                                  in_=cen_b[:, base + 67:base + 70, :])

                # ---- x conv: x[16g+c, 3+r, XP+w] = w_in . cen(base+8g+r)
                # x is bf16 so the contrast stage runs DVE 2x_1P mode; x_odd
                # is x shifted left one column so odd-dw reads stay 4B-aligned
                x = xpool.tile([C, XR, XW], BF, tag="x")
                nc.gpsimd.memset(x[:, :, 0:XP], 0.0)
                nc.gpsimd.memset(x[:, :, W + XP:XW], 0.0)

                for r in range(GR):
                    pxc = pspool.tile([C, W], F32, tag="ps")
                    for g in range(G):
                        nc.tensor.matmul(
                            pxc[:], cbf_sb[:, g * C:(g + 1) * C],
                            cg[g][:, r, :],
                            start=(g == 0), stop=(g == G - 1))
                    if R_XW_DVE:
                        nc.vector.tensor_scalar_add(
                            x[:, 3 + r, XP: XP + W], pxc[:],
                            cf32_sb[:, 0:1])
                    else:
                        nc.scalar.activation(
                            x[:, 3 + r, XP: XP + W], pxc[:], AF.Identity,
                            bias=cf32_sb[:, 0:1], scale=1.0)

                for j in range(3):
                    pe = pspool.tile([C, W], F32, tag="ps")
                    nc.tensor.matmul(pe[:], cbf_sb[:, 0:C],
                                     et[:, j, :], start=True,
                                     stop=True)
                    bt_c = 1 if mh == 0 else 0
                    nc.scalar.activation(
                        x[0:CR, j, XP: XP + W], pe[0:CR, :], AF.Identity,
                        bias=cf32_sb[0:CR, bt_c:bt_c + 1], scale=1.0)
                    pe2 = pspool.tile([C, W], F32, tag="ps")
                    nc.tensor.matmul(pe2[:], cbf_sb[:, (G - 1) * C:G * C],
                                     eb[:, j, :], start=True,
                                     stop=True)
                    # start partition must be a multiple of 32; rows 96:112 get
                    # junk here and are re-written by the interior-halo DMA
                    # below (WAW-ordered by Tile).
                    bb_c = 2 if mh == MH - 1 else 0
                    nc.scalar.activation(
                        x[96: C, 11 + j, XP: XP + W], pe2[96: C, :],
                        AF.Identity, bias=cf32_sb[96:C, bb_c:bb_c + 1], scale=1.0)

                # interior halos between groups via partition-shifted SBUF DMA
                nc.sync.dma_start(out=x[CR:C, 0:3, XP:XP + W],
                                  in_=x[0:C - CR, GR:GR + 3, XP:XP + W])
                nc.sync.dma_start(out=x[0:C - CR, GR + 3:GR + 6, XP:XP + W],
                                  in_=x[CR:C, 3:6, XP:XP + W])

                # ---- mas path: 3x3 conv via 9 shifted loads on 72 partitions
                # mas_p is column-padded so every row read is a full 512B run
                mas9 = mas9pool.tile([72, GR, W], BF, tag="m9")
                t = 0
                for dy in (-1, 0, 1):
                    for dx in (-1, 0, 1):
                        msrc = mas_p[base + dy + 1: base + dy + 1 + 64,
                                     1 + dx: 1 + dx + W]
                        msrc = msrc.rearrange("(g x) w -> g x w", x=GR)
                        nc.sync.dma_start(out=mas9[G * t:G * (t + 1), :, :],
                                          in_=msrc)
                        t += 1
                # mm_q rows 0:8 = sigmoid(mas conv path); row 8 = ones (for
                # the s3 constant folded into the gate broadcast matmul)
                mm_q = qmpool.tile([9, QF], BF, tag="mmq")
                nc.gpsimd.memset(mm_q[:, :], 1.0)
                m9f = _fl(mas9)
                for c2 in range(4):
                    cs2 = slice(512 * c2, 512 * (c2 + 1))
                    pm = pupool.tile([G, 512], F32, tag="pu")
                    nc.tensor.matmul(pm[:], cbf_sb[0:72, CB_MAS:CB_MAS + 8],
                                     m9f[:, cs2], start=True, stop=True)
                    m_t = masspool.tile([G, 512], BF, tag="qs")
                    nc.scalar.activation(m_t[:], pm[:], AF.Identity,
                                         bias=sc(S_MB1, G), scale=1.0)
                    m_s = masspool.tile([G, 512], BF, tag="qs")
                    nc.scalar.activation(m_s[:], m_t[:], AF.Sigmoid)
                    msil = masspool.tile([G, 512], BF, tag="qs")
                    (nc.gpsimd if R_MSIL_POOL else nc.vector).tensor_tensor(
                        msil[:], m_t[:], m_s[:], ALU.mult)
                    nc.scalar.activation(mm_q[0:G, cs2], msil[:],
                                         AF.Sigmoid, bias=sc(S_MB2, G),
                                         scale=sc(S_MW2, G))

                # x_odd[c] = x[c+1], one flat shifted copy (pads included)
                x_odd = xopool.tile([C, XR, XW], BF, tag="xo")
                xf = _fl(x)
                xof = _fl(x_odd)
                nc.vector.tensor_copy(xof[:, 0:XR * XW - 1], xf[:, 1:XR * XW])

                j0 = 3

                def xin(dh, dw):
                    if dw % 2 == 0:
                        return x[:, j0 + dh:j0 + GR + dh,
                                 XP + dw:XP + W + dw]
                    return x_odd[:, j0 + dh:j0 + GR + dh,
                                 XP + dw - 1:XP + W + dw - 1]

                # ---- contrast stage ----
                ts_tiles = []
                for s in (1, 3):
                    dirs = [(-s, -s), (-s, 0), (-s, s), (0, -s)]
                    o_t = []
                    for (dh, dw) in dirs:
                        d1 = dpool.tile([C, GR, W], BF, tag="d")
                        nc.vector.tensor_tensor(
                            d1[:], x[:, j0:j0 + GR, XP:XP + W],
                            xin(dh, dw), ALU.subtract)
                        d2 = dpool.tile([C, GR, W], BF, tag="d")
                        nc.vector.tensor_tensor(
                            d2[:], x[:, j0:j0 + GR, XP:XP + W],
                            xin(-dh, -dw), ALU.subtract)
                        o = opool.tile([C, GR, W], BF, tag="o")
                        nc.vector.tensor_tensor(_fl(o), _fl(d1), _fl(d2),
                                                ALU.mult)
                        o_t.append(o)

                    def tree(op, eng):
                        a = trpool.tile([C, QF], BF, tag="tr")
                        eng.tensor_tensor(a[:], _fl(o_t[0]),
                                          _fl(o_t[1]), op)
                        b = trpool.tile([C, QF], BF, tag="tr")
                        eng.tensor_tensor(b[:], _fl(o_t[2]),
                                          _fl(o_t[3]), op)
                        r_ = trpool.tile([C, QF], BF, tag="tr")
                        eng.tensor_tensor(r_[:], a[:], b[:], op)
                        return r_

                    min4 = tree(ALU.min, nc.vector)
                    max4 = tree(ALU.max, nc.vector)
                    sum4 = tree(ALU.add,
                                nc.gpsimd if R_SUM_POOL else nc.vector)

                    def scale_op(src, col):
                        q = trpool.tile([C, QF], BF, tag="tr")
                        if R_TSP_ACT:
                            nc.scalar.activation(q[:], src[:], AF.Identity,
                                                 scale=sc(col))
                        else:
                            nc.vector.tensor_scalar_mul(q[:], src[:], sc(col))
                        return q

                    q1 = scale_op(sum4, S_W1_4)
                    qa = scale_op(min4, S_W0)
                    qb = scale_op(max4, S_W2)
                    qu = trpool.tile([C, QF], BF, tag="tr")
                    nc.vector.tensor_tensor(qu[:], qa[:], qb[:], ALU.add)
                    t_s = tspool.tile([C, QF], BF, tag="ts")
                    nc.vector.tensor_tensor(t_s[:], qu[:], q1[:], ALU.add)
                    ts_tiles.append(t_s)

                t1, t3 = ts_tiles
                mn = typool.tile([C, QF], BF, tag="tymn")
                nc.vector.tensor_tensor(mn[:], t1[:], t3[:], ALU.min)
                mx = typool.tile([C, QF], BF, tag="tymx")
                nc.vector.tensor_tensor(mx[:], t1[:], t3[:], ALU.max)

                # ---- bc 1x1 conv + BN + SiLU (bf16 z path) ----
                # min+max == sum for two elements, so the scale2-weighted
                # combine needs only two accumulating matmul terms with
                # pre-scaled bc weights: (v0+v1/2)*mn + (v2+v1/2)*mx
                t_bn = zpool.tile([C, QF], BF, tag="tbn")
                for h2 in range(2):
                    pbc = pbpool.tile([C, 1024], F32, tag="pb")
                    for c2 in range(2):
                        lo = 1024 * h2 + 512 * c2
                        cc = slice(512 * c2, 512 * (c2 + 1))
                        nc.tensor.matmul(pbc[:, cc],
                                         cbf_sb[:, CB_BC:CB_BC + C],
                                         mn[:, lo:lo + 512],
                                         start=True, stop=False)
                        nc.tensor.matmul(pbc[:, cc],
                                         cbf_sb[:, CB_BC + C:CB_BC + 2 * C],
                                         mx[:, lo:lo + 512],
                                         start=False, stop=True)
                    nc.scalar.activation(t_bn[:, 1024 * h2:1024 * (h2 + 1)],
                                         pbc[:], AF.Identity,
                                         bias=cf32_sb[:, 4:5],
                                         scale=cf32_sb[:, 3:4])
                sg = zpool.tile([C, QF], BF, tag="sg")
                nc.scalar.activation(sg[:], t_bn[:], AF.Sigmoid)
                z_q = zpool.tile([C, QF], BF, tag="z")
                nc.vector.tensor_tensor(z_q[:], t_bn[:], sg[:], ALU.mult)

                # ---- w_out 1x1 + sigmoid -> om ----
                om_q = qmpool.tile([G, QF], BF, tag="omq")
                for c2 in range(4):
                    pu = pupool.tile([G, 512], F32, tag="pu")
                    nc.tensor.matmul(pu[:], cbf_sb[:, CB_WOUT:CB_WOUT + 8],
                                     z_q[:, 512 * c2:512 * (c2 + 1)],
                                     start=True, stop=True)
                    nc.scalar.activation(om_q[:, 512 * c2:512 * (c2 + 1)],
                                         pu[:], AF.Sigmoid,
                                         bias=sc(S_BOUT, G), scale=1.0)

                # ---- gate: ct = om*(s2*mm+s0); linear terms s1*mm + s3
                # folded into the broadcast matmul (bcast_mm lhsT) ----
                bt = dpool.tile([G, QF], BF, tag="d")
                nc.vector.tensor_scalar(bt[:], mm_q[0:G, :], sc(S_G2, G),
                                        sc(S_G0, G), ALU.mult, ALU.add)
                ct = qcpool.tile([G, QF], BF, tag="ctq")
                nc.vector.tensor_tensor(ct[:], om_q[:], bt[:], ALU.mult)

                # ---- broadcast gate + final multiply ----
                # cen is re-read from DRAM into transient tiles here so the
                # cg tiles release right after the x-conv, letting the next
                # half's cen loads and x-conv overlap this half's tail.
                for g in range(G):
                    cf = cfpool.tile([C, GR, W], BF, tag="cf")
                    nc.scalar.dma_start(
                        out=cf[:],
                        in_=cen_b[:, base + 3 + 8 * g: base + 3 + 8 * g + 8, :])
                    out_t = outpool.tile([C, GR, W], BF, tag="ot")
                    for h2 in range(2):
                        pg = pbpool.tile([C, 1024], F32, tag="pb")
                        for c2 in range(2):
                            lo = 1024 * h2 + 512 * c2
                            cc = slice(512 * c2, 512 * (c2 + 1))
                            nc.tensor.matmul(
                                pg[:, cc],
                                cbf_sb[0:8, CB_BCT + g * C:CB_BCT + (g + 1) * C],
                                ct[:, lo:lo + 512],
                                start=True, stop=False)
                            nc.tensor.matmul(
                                pg[:, cc],
                                cbf_sb[0:9, CB_BMM + g * C:CB_BMM + (g + 1) * C],
                                mm_q[:, lo:lo + 512],
                                start=False, stop=True)
                        cen_fl = cf[:, 4 * h2:4 * h2 + 4, :].rearrange(
                            "p a b -> p (a b)")
                        out_fl = out_t[:, 4 * h2:4 * h2 + 4, :].rearrange(
                            "p a b -> p (a b)")
                        gb = gbpool.tile([C, 1024], BF, tag="gb")
                        nc.scalar.copy(gb[:], pg[:])
                        fm_eng = (nc.gpsimd if (R_FM_POOL and g % 2 == 1)
                                  else nc.vector)
                        fm_eng.tensor_tensor(out_fl, cen_fl,
                                             gb[:], ALU.mult)
                    nc.scalar.dma_start(
                        out=out_p[:, base + 8 * g:base + 8 * g + 8, :],
                        in_=out_t[:])

            rep_ctx = (tc.For_i(0, loop_reps, 1) if loop_reps
                       else contextlib.nullcontext())
            with rep_ctx:
                for mh in range(MH):
                    emit_half(mh)
    nc.compile()
    return nc


def _softmax(v):
    e = np.exp(v - v.max())
    return e / e.sum()


def _prep_consts(inp):
    w = _softmax(inp['scale1'])
    v = _softmax(inp['scale2'])
    s3 = _softmax(inp['scale3'])
    inv = inp['bn_gamma'] / np.sqrt(inp['bn_var'] + BN_EPS)
    bnb = inp['bn_beta'] - inp['bn_mean'] * inv

    w_in_blk = np.zeros((C, G, C), np.float32)
    for g in range(G):
        w_in_blk[:, g, CR * g:CR * (g + 1)] = inp['w_in'].T
    b_in_t = np.tile(inp['b_in'], G)[:, None].astype(np.float32)

    bc_blk = np.kron(np.eye(G), inp['bc_w'].T)
    a_w = v[0] + v[1] / 2.0
    b_w = v[2] + v[1] / 2.0
    bc2 = np.stack([a_w * bc_blk, b_w * bc_blk], 1)  # [C, 2, C]

    wout_lhsT = np.kron(np.eye(G), inp['w_out'][0][:, None]).astype(np.float32)
    k_flat = inp['mas_w1'][0, 0].reshape(9)
    mas_lhsT = np.kron(k_flat[:, None], np.eye(G)).astype(np.float32)

    bcast_ct = np.zeros((8, G, C), np.float32)
    for g in range(G):
        bcast_ct[g, g, :] = 1.0
    bcast_mm = np.zeros((9, G, C), np.float32)
    for g in range(G):
        bcast_mm[g, g, :] = s3[1]
    bcast_mm[8, :, :] = s3[3]

    scal = np.zeros((C, 13), np.float32)
    vals = [w[0], w[1] / 4.0, w[2], v[0], v[1] / 2.0, v[2],
            s3[0], s3[1], s3[2], inp['b_out'][0], inp['mas_b1'][0],
            inp['mas_w2'][0, 0], inp['mas_b2'][0]]
    scal[:] = np.asarray(vals, np.float32)[None, :]

    cbf = np.zeros((C, CB_W), np.float32)
    cbf[:, CB_WIN:CB_WIN + G * C] = w_in_blk.reshape(C, G * C)
    cbf[:, CB_BC:CB_BC + 2 * C] = bc2.reshape(C, 2 * C)
    cbf[:, CB_WOUT:CB_WOUT + 8] = wout_lhsT
    cbf[0:72, CB_MAS:CB_MAS + 8] = mas_lhsT
    cbf[0:8, CB_BCT:CB_BCT + G * C] = bcast_ct.reshape(8, G * C)
    cbf[0:9, CB_BMM:CB_BMM + G * C] = bcast_mm.reshape(9, G * C)
    cf32 = np.zeros((C, 18), np.float32)
    cf32[:, 0:1] = b_in_t
    cf32[:, 3:4] = np.tile(inv, G)[:, None].astype(np.float32)
    cf32[:, 4:5] = np.tile(bnb, G)[:, None].astype(np.float32)
    cf32[:, 5:18] = scal
    return {'cbf': cbf.astype(ml_dtypes.bfloat16), 'cf32': cf32,
            'b_in_t': b_in_t}


def _core_inputs(inp, consts, core):
    b, hf = core // 2, core % 2
    r0 = CH * hf
    cen_pad = np.pad(inp['cen'][b], ((0, 0), (3, 3), (0, 0)))
    mas_pad = np.pad(inp['mas'][b, 0], ((1, 9), (1, 1)))
    cen_core = np.ascontiguousarray(cen_pad[:, r0:r0 + CH + 6, :])
    cf32 = consts['cf32'].copy()
    if hf != 0:
        cf32[:, 1:2] = consts['b_in_t']
    if hf != 1:
        cf32[:, 2:3] = consts['b_in_t']
    return {
        'cen_bf': cen_core.astype(ml_dtypes.bfloat16),
        'mas': np.ascontiguousarray(
            mas_pad[r0:r0 + CH + 8, :]).astype(ml_dtypes.bfloat16),
        'cbf': consts['cbf'],
        'cf32': cf32,
    }


def run(inputs, trace=False):
    inp = {k: np.asarray(v) for k, v in inputs.items()}
    consts = _prep_consts(inp)

    if 'nc' not in _CACHE:
        _CACHE['nc'] = build_nc()
    nc = _CACHE['nc']

    in_maps = [_core_inputs(inp, consts, core) for core in range(N_CORES)]
    res = run_bass_kernel_spmd(nc, in_maps, list(range(N_CORES)), trace=trace)

    out = np.empty((4, C, H, W), np.float32)
    for core in range(N_CORES):
        b, hf = core // 2, core % 2
        out[b, :, CH * hf:CH * (hf + 1), :] = res.results[core]['out'].astype(
            np.float32)
    return out, res


def kernel(**inputs):
    return run(inputs)[0]


def bench(inputs, iters=30, reps=0):
    """Time repeated executions with device-resident inputs (no donation).

    Returns (out, per_call_seconds_list). The kernel writes every output
    element, so dropping the zero-buffer donation is safe.
    """
    import time
    import jax
    from jax.sharding import Mesh, PartitionSpec
    from jax.experimental.shard_map import shard_map
    from concourse import bass2jax

    inp = {k: np.asarray(v) for k, v in inputs.items()}
    consts = _prep_consts(inp)
    key = ('nc', reps)
    if key not in _CACHE:
        _CACHE[key] = build_nc(loop_reps=reps)
    nc = _CACHE[key]

    in_maps = [_core_inputs(inp, consts, core) for core in range(N_CORES)]

    bass2jax.install_neuronx_cc_hook()
    in_names, out_names, out_avals, zero_outs = [], [], [], []
    for alloc in nc.m.functions[0].allocations:
        if not isinstance(mybir.MemoryLocationSet, type) or not isinstance(
                alloc, mybir.MemoryLocationSet):
            continue
        name = alloc.memorylocations[0].name
        pname = (nc.partition_id_tensor.name if nc.partition_id_tensor
                 else None)
        if alloc.kind == "ExternalInput":
            if name != pname:
                in_names.append(name)
        elif alloc.kind == "ExternalOutput":
            out_names.append(name)
            out_avals.append(jax.core.ShapedArray(
                tuple(alloc.tensor_shape), mybir.dt.np(alloc.dtype)))
            zero_outs.append(np.zeros(tuple(alloc.tensor_shape),
                                      mybir.dt.np(alloc.dtype)))
    n_params = len(in_names)
    all_names = in_names + out_names
    if nc.partition_id_tensor:
        all_names = all_names + [nc.partition_id_tensor.name]

    def _body(*args):
        operands = list(args)
        if nc.partition_id_tensor:
            operands.append(bass2jax.partition_id_tensor())
        outs = bass2jax._bass_exec_p.bind(
            *operands,
            out_avals=tuple(out_avals),
            in_names=tuple(all_names),
            out_names=tuple(out_names),
            lowering_input_output_aliases=(),
            sim_require_finite=True,
            sim_require_nnan=True,
            nc=nc,
        )
        return tuple(outs)

    devices = jax.devices()[:N_CORES]
    mesh = Mesh(np.asarray(devices), ("core",))
    nin = n_params + len(out_names)
    sharded = jax.jit(
        shard_map(_body, mesh=mesh,
                  in_specs=(PartitionSpec("core"),) * nin,
                  out_specs=(PartitionSpec("core"),) * len(out_names),
                  check_rep=False),
        donate_argnums=tuple(range(n_params, n_params + len(out_names))),
        keep_unused=True,
    )
    concat_in = [np.concatenate([in_maps[c][nm] for c in range(N_CORES)], 0)
                 for nm in in_names]
    concat_zero = [np.zeros((N_CORES * z.shape[0], *z.shape[1:]), z.dtype)
                   for z in zero_outs]
    sh = jax.sharding.NamedSharding(mesh, PartitionSpec("core"))
    dev_in = [jax.device_put(a, sh) for a in concat_in]
    prev = jax.device_put(concat_zero[0], sh)

    outs = sharded(*dev_in, prev)
    jax.block_until_ready(outs)
    result = np.asarray(outs[0]).copy()
    prev = outs[0]
    times = []
    for _ in range(iters):
        t0 = time.perf_counter()
        outs = sharded(*dev_in, prev)
        jax.block_until_ready(outs)
        times.append(time.perf_counter() - t0)
        prev = outs[0]

    full = np.empty((4, C, H, W), np.float32)
    arr = result.reshape(N_CORES, C, CH, W)
    for core in range(N_CORES):
        b, hf = core // 2, core % 2
        full[b, :, CH * hf:CH * (hf + 1), :] = arr[core].astype(np.float32)
    return full, times


# revision 24
# speedup vs baseline: 1.1397x; 1.1397x over previous
"""Trainium2 Bass kernel for nn_ExpansionContrastModule.

Strategy: pure data parallel over 8 cores; each core processes half of one
batch image (128 of 256 rows), with a 3-row halo so the dilated contrast
convs and the 3x3 mas conv need no cross-core traffic.

v4 (HW-measured 201us/rep vs 246us for the f32 baseline):
- cen is read once as bf16 for the x-conv (per-group cg tiles, released
  right after the x-conv) and re-read bf16 for the final multiply via
  transient cf tiles, so consecutive halves pipeline instead of
  serializing on cen residency.
- output is written bf16 and converted to f32 on the host.
- prologue loads (cg/edges/mas) issue on the nc.sync HWDGE ring; tail
  DMAs (cf loads, out writes) on the nc.scalar ring, so the next rep's
  input loads never queue behind this rep's tail writes.
- mas input is column-padded to 258 so every mas9 row read is a full
  512B run (no <512B DMA penalty, no edge memsets).
- the two-shift min/mean/max combine is folded to a*min + b*max
  (min+max == sum for two elements) -> 2 bc matmul terms instead of 3.
- the gate's linear terms (s1*mm + s3) are folded into the PE broadcast
  matmul via an extra ones-row; only ct = om*(s2*mm+s0) needs the DVE.
- elementwise work stays on DVE (bf16 2x) with the sum trees included:
  offloading chains to Pool measured 3-5x worse than the cost model.
"""
import sys
import ml_dtypes
import numpy as np

sys.path.insert(0, "/opt/trn_rl_repo")

import concourse.bass as bass
import concourse.bacc as bacc
import concourse.mybir as mybir
from concourse.tile import TileContext
from concourse.bass_utils import run_bass_kernel_spmd

F32 = mybir.dt.float32
AF = mybir.ActivationFunctionType
ALU = mybir.AluOpType

N_CORES = 8
C = 128        # input channels
CR = 16        # reduced channels
H = W = 256
CH = 128       # rows per core (half an image)
MH = 2         # macro-halves per core
HB = 64        # rows per macro-half
G = 8          # row-groups per macro-half
GR = 8         # rows per group
XR = GR + 6    # x tile rows (3-row halo each side)
XP = 4         # x tile left/right col pad (4 for bf16 4B alignment)
XW = W + 2 * XP  # x tile cols
QF = GR * W    # free elems per slab (2048)
BF = mybir.dt.bfloat16

BN_EPS = 1e-5

# cbf column layout
CB_WIN = 0          # w_in_blk       [C, 8*C]
CB_BC = 1024        # bc2 (a,b)      [C, 2*C]
CB_WOUT = 1280      # wout_lhsT      [C, 8]
CB_MAS = 1288       # mas_lhsT       [72, 8]
CB_BCT = 1296       # bcast_ct       [8, 8*C]
CB_BMM = 2320       # bcast_mm       [9, 8*C]
CB_W = 3344

# cf32/scal column indices
S_W0, S_W1_4, S_W2, S_V0, S_V1_2, S_V2 = 0, 1, 2, 3, 4, 5
S_G0, S_G1, S_G2, S_BOUT, S_MB1, S_MW2, S_MB2 = 6, 7, 8, 9, 10, 11, 12

_CACHE = {}

# elementwise routing knobs (bisect HW vs cost-model behavior)
R_SUM_POOL = False    # sum tree on Pool engine instead of DVE
R_TSP_ACT = False     # q1/qa/qb scale ops on ACT instead of DVE
R_MSIL_POOL = False   # mas silu multiply on Pool
R_FM_POOL = False     # alternate final-mult chunks onto Pool (measured slower)
R_XW_DVE = False      # interior x-row writes via DVE (measured slower)


def _fl(t):
    """Flatten the two free dims of a [P, a, b] tile AP into [P, a*b]."""
    return t[:, :, :].rearrange("p a b -> p (a b)")


def build_nc(loop_reps=0):
    nc = bacc.Bacc("TRN2", target_bir_lowering=False, debug=False,
                   num_devices=N_CORES)
    cen_b = nc.dram_tensor("cen_bf", [C, CH + 6, W], BF, kind="ExternalInput")
    mas_p = nc.dram_tensor("mas", [CH + 8, W + 2], BF, kind="ExternalInput")
    cbf_d = nc.dram_tensor("cbf", [C, CB_W], BF, kind="ExternalInput")
    cf32_d = nc.dram_tensor("cf32", [C, 18], F32, kind="ExternalInput")
    out_p = nc.dram_tensor("out", [C, CH, W], BF, kind="ExternalOutput")

    with TileContext(nc) as tc:
        import contextlib
        _stk = contextlib.ExitStack()
        with _stk:
            cpool = _stk.enter_context(tc.tile_pool(name="const", bufs=1))
            cgpool = _stk.enter_context(tc.tile_pool(name="cg", bufs=9))
            cepool = _stk.enter_context(tc.tile_pool(name="ce", bufs=1))
            xpool = _stk.enter_context(tc.tile_pool(name="x", bufs=2))
            xopool = _stk.enter_context(tc.tile_pool(name="xo", bufs=2))
            dpool = _stk.enter_context(tc.tile_pool(name="d", bufs=3))
            opool = _stk.enter_context(tc.tile_pool(name="o", bufs=4))
            trpool = _stk.enter_context(tc.tile_pool(name="tr", bufs=5))
            tspool = _stk.enter_context(tc.tile_pool(name="tsb", bufs=2))
            typool = _stk.enter_context(tc.tile_pool(name="ty", bufs=1))
            zpool = _stk.enter_context(tc.tile_pool(name="z", bufs=1))
            qmpool = _stk.enter_context(tc.tile_pool(name="qm", bufs=2))
            qcpool = _stk.enter_context(tc.tile_pool(name="qc", bufs=2))
            mas9pool = _stk.enter_context(tc.tile_pool(name="m9", bufs=2))
            masspool = _stk.enter_context(tc.tile_pool(name="ms", bufs=4))
            gbpool = _stk.enter_context(tc.tile_pool(name="gb", bufs=2))
            cfpool = _stk.enter_context(tc.tile_pool(name="cf", bufs=2))
            outpool = _stk.enter_context(tc.tile_pool(name="ot", bufs=2))
            pspool = _stk.enter_context(tc.tile_pool(name="ps", bufs=2, space="PSUM"))
            pbpool = _stk.enter_context(tc.tile_pool(name="pb", bufs=2, space="PSUM"))
            pupool = _stk.enter_context(tc.tile_pool(name="pu", bufs=2, space="PSUM"))
            # ---- constants ----
            cbf_sb = cpool.tile([C, CB_W], BF, tag="c_bf")
            cf32_sb = cpool.tile([C, 18], F32, tag="c_f32")
            nc.sync.dma_start(out=cbf_sb[:], in_=cbf_d[:])
            nc.sync.dma_start(out=cf32_sb[:], in_=cf32_d[:])

            def sc(col, p=C):
                return cf32_sb[0:p, 5 + col:6 + col]

            def emit_half(mh):
                base = mh * HB

                # ---- load this half's cen rows as 8 per-group tiles
                # (cg[g] row r = image row r0+base+8g+r); kept resident: the
                # x-conv matmuls and the final multiply both read them, and
                # each tile releases right after its group's final multiply.
                cg = []
                for g in range(G):
                    cg_t = cgpool.tile([C, GR, W], BF, tag="cg")
                    nc.sync.dma_start(
                        out=cg_t[:],
                        in_=cen_b[:, base + 3 + 8 * g: base + 3 + 8 * g + 8, :])
                    cg.append(cg_t)
                et = cepool.tile([C, 3, W], BF, tag="et")
                nc.sync.dma_start(out=et[:], in_=cen_b[:, base:base + 3, :])
                eb = cepool.tile([C, 3, W], BF, tag="eb")
                nc.sync.dma_start(out=eb[:],
                                  in_=cen_b[:, base + 67:base + 70, :])

                # ---- x conv: x[16g+c, 3+r, XP+w] = w_in . cen(base+8g+r)
                # x is bf16 so the contrast stage runs DVE 2x_1P mode; x_odd
                # is x shifted left one column so odd-dw reads stay 4B-aligned
                x = xpool.tile([C, XR, XW], BF, tag="x")
                nc.gpsimd.memset(x[:, :, 0:XP], 0.0)
                nc.gpsimd.memset(x[:, :, W + XP:XW], 0.0)

                for r in range(GR):
                    pxc = pspool.tile([C, W], F32, tag="ps")
                    for g in range(G):
                        nc.tensor.matmul(
                            pxc[:], cbf_sb[:, g * C:(g + 1) * C],
                            cg[g][:, r, :],
                            start=(g == 0), stop=(g == G - 1))
                    if R_XW_DVE:
                        nc.vector.tensor_scalar_add(
                            x[:, 3 + r, XP: XP + W], pxc[:],
                            cf32_sb[:, 0:1])
                    else:
                        nc.scalar.activation(
                            x[:, 3 + r, XP: XP + W], pxc[:], AF.Identity,
                            bias=cf32_sb[:, 0:1], scale=1.0)

                for j in range(3):
                    pe = pspool.tile([C, W], F32, tag="ps")
                    nc.tensor.matmul(pe[:], cbf_sb[:, 0:C],
                                     et[:, j, :], start=True,
                                     stop=True)
                    bt_c = 1 if mh == 0 else 0
                    nc.scalar.activation(
                        x[0:CR, j, XP: XP + W], pe[0:CR, :], AF.Identity,
                        bias=cf32_sb[0:CR, bt_c:bt_c + 1], scale=1.0)
                    pe2 = pspool.tile([C, W], F32, tag="ps")
                    nc.tensor.matmul(pe2[:], cbf_sb[:, (G - 1) * C:G * C],
                                     eb[:, j, :], start=True,
                                     stop=True)
                    # start partition must be a multiple of 32; rows 96:112 get
                    # junk here and are re-written by the interior-halo DMA
                    # below (WAW-ordered by Tile).
                    bb_c = 2 if mh == MH - 1 else 0
                    nc.scalar.activation(
                        x[96: C, 11 + j, XP: XP + W], pe2[96: C, :],
                        AF.Identity, bias=cf32_sb[96:C, bb_c:bb_c + 1], scale=1.0)

                # interior halos between groups via partition-shifted SBUF DMA
                nc.sync.dma_start(out=x[CR:C, 0:3, XP:XP + W],
                                  in_=x[0:C - CR, GR:GR + 3, XP:XP + W])
                nc.sync.dma_start(out=x[0:C - CR, GR + 3:GR + 6, XP:XP + W],
                                  in_=x[CR:C, 3:6, XP:XP + W])

                # ---- mas path: 3x3 conv via 9 shifted loads on 72 partitions
                # mas_p is column-padded so every row read is a full 512B run
                mas9 = mas9pool.tile([72, GR, W], BF, tag="m9")
                t = 0
                for dy in (-1, 0, 1):
                    for dx in (-1, 0, 1):
                        msrc = mas_p[base + dy + 1: base + dy + 1 + 64,
                                     1 + dx: 1 + dx + W]
                        msrc = msrc.rearrange("(g x) w -> g x w", x=GR)
                        nc.sync.dma_start(out=mas9[G * t:G * (t + 1), :, :],
                                          in_=msrc)
                        t += 1
                # mm_q rows 0:8 = sigmoid(mas conv path); row 8 = ones (for
                # the s3 constant folded into the gate broadcast matmul)
                mm_q = qmpool.tile([9, QF], BF, tag="mmq")
                nc.gpsimd.memset(mm_q[:, :], 1.0)
                m9f = _fl(mas9)
                for c2 in range(4):
                    cs2 = slice(512 * c2, 512 * (c2 + 1))
                    pm = pupool.tile([G, 512], F32, tag="pu")
                    nc.tensor.matmul(pm[:], cbf_sb[0:72, CB_MAS:CB_MAS + 8],
                                     m9f[:, cs2], start=True, stop=True)
                    m_t = masspool.tile([G, 512], BF, tag="qs")
                    nc.scalar.activation(m_t[:], pm[:], AF.Identity,
                                         bias=sc(S_MB1, G), scale=1.0)
                    m_s = masspool.tile([G, 512], BF, tag="qs")
                    nc.scalar.activation(m_s[:], m_t[:], AF.Sigmoid)
                    msil = masspool.tile([G, 512], BF, tag="qs")
                    (nc.gpsimd if R_MSIL_POOL else nc.vector).tensor_tensor(
                        msil[:], m_t[:], m_s[:], ALU.mult)
                    nc.scalar.activation(mm_q[0:G, cs2], msil[:],
                                         AF.Sigmoid, bias=sc(S_MB2, G),
                                         scale=sc(S_MW2, G))

                # x_odd[c] = x[c+1], one flat shifted copy (pads included)
                x_odd = xopool.tile([C, XR, XW], BF, tag="xo")
                xf = _fl(x)
                xof = _fl(x_odd)
                nc.vector.tensor_copy(xof[:, 0:XR * XW - 1], xf[:, 1:XR * XW])

                j0 = 3

                def xin(dh, dw):
                    if dw % 2 == 0:
                        return x[:, j0 + dh:j0 + GR + dh,
                                 XP + dw:XP + W + dw]
                    return x_odd[:, j0 + dh:j0 + GR + dh,
                                 XP + dw - 1:XP + W + dw - 1]

                # ---- contrast stage ----
                ts_tiles = []
                for s in (1, 3):
                    dirs = [(-s, -s), (-s, 0), (-s, s), (0, -s)]
                    o_t = []
                    for (dh, dw) in dirs:
                        d1 = dpool.tile([C, GR, W], BF, tag="d")
                        nc.vector.tensor_tensor(
                            d1[:], x[:, j0:j0 + GR, XP:XP + W],
                            xin(dh, dw), ALU.subtract)
                        d2 = dpool.tile([C, GR, W], BF, tag="d")
                        nc.vector.tensor_tensor(
                            d2[:], x[:, j0:j0 + GR, XP:XP + W],
                            xin(-dh, -dw), ALU.subtract)
                        o = opool.tile([C, GR, W], BF, tag="o")
                        nc.vector.tensor_tensor(_fl(o), _fl(d1), _fl(d2),
                                                ALU.mult)
                        o_t.append(o)

                    def tree(op, eng):
                        a = trpool.tile([C, QF], BF, tag="tr")
                        eng.tensor_tensor(a[:], _fl(o_t[0]),
                                          _fl(o_t[1]), op)
                        b = trpool.tile([C, QF], BF, tag="tr")
                        eng.tensor_tensor(b[:], _fl(o_t[2]),
                                          _fl(o_t[3]), op)
                        r_ = trpool.tile([C, QF], BF, tag="tr")
                        eng.tensor_tensor(r_[:], a[:], b[:], op)
                        return r_

                    min4 = tree(ALU.min, nc.vector)
                    max4 = tree(ALU.max, nc.vector)
                    sum4 = tree(ALU.add,
                                nc.gpsimd if R_SUM_POOL else nc.vector)

                    def scale_op(src, col):
                        q = trpool.tile([C, QF], BF, tag="tr")
                        if R_TSP_ACT:
                            nc.scalar.activation(q[:], src[:], AF.Identity,
                                                 scale=sc(col))
                        else:
                            nc.vector.tensor_scalar_mul(q[:], src[:], sc(col))
                        return q

                    q1 = scale_op(sum4, S_W1_4)
                    qa = scale_op(min4, S_W0)
                    qb = scale_op(max4, S_W2)
                    qu = trpool.tile([C, QF], BF, tag="tr")
                    nc.vector.tensor_tensor(qu[:], qa[:], qb[:], ALU.add)
                    t_s = tspool.tile([C, QF], BF, tag="ts")
                    nc.vector.tensor_tensor(t_s[:], qu[:], q1[:], ALU.add)
                    ts_tiles.append(t_s)

                t1, t3 = ts_tiles
                mn = typool.tile([C, QF], BF, tag="tymn")
                nc.vector.tensor_tensor(mn[:], t1[:], t3[:], ALU.min)
                mx = typool.tile([C, QF], BF, tag="tymx")
                nc.vector.tensor_tensor(mx[:], t1[:], t3[:], ALU.max)

                # ---- bc 1x1 conv + BN + SiLU (bf16 z path) ----
                # min+max == sum for two elements, so the scale2-weighted
                # combine needs only two accumulating matmul terms with
                # pre-scaled bc weights: (v0+v1/2)*mn + (v2+v1/2)*mx
                t_bn = zpool.tile([C, QF], BF, tag="tbn")
                for h2 in range(2):
                    pbc = pbpool.tile([C, 1024], F32, tag="pb")
                    for c2 in range(2):
                        lo = 1024 * h2 + 512 * c2
                        cc = slice(512 * c2, 512 * (c2 + 1))
                        nc.tensor.matmul(pbc[:, cc],
                                         cbf_sb[:, CB_BC:CB_BC + C],
                                         mn[:, lo:lo + 512],
                                         start=True, stop=False)
                        nc.tensor.matmul(pbc[:, cc],
                                         cbf_sb[:, CB_BC + C:CB_BC + 2 * C],
                                         mx[:, lo:lo + 512],
                                         start=False, stop=True)
                    nc.scalar.activation(t_bn[:, 1024 * h2:1024 * (h2 + 1)],
                                         pbc[:], AF.Identity,
                                         bias=cf32_sb[:, 4:5],
                                         scale=cf32_sb[:, 3:4])
                sg = zpool.tile([C, QF], BF, tag="sg")
                nc.scalar.activation(sg[:], t_bn[:], AF.Sigmoid)
                z_q = zpool.tile([C, QF], BF, tag="z")
                nc.vector.tensor_tensor(z_q[:], t_bn[:], sg[:], ALU.mult)

                # ---- w_out 1x1 + sigmoid -> om ----
                om_q = qmpool.tile([G, QF], BF, tag="omq")
                for c2 in range(4):
                    pu = pupool.tile([G, 512], F32, tag="pu")
                    nc.tensor.matmul(pu[:], cbf_sb[:, CB_WOUT:CB_WOUT + 8],
                                     z_q[:, 512 * c2:512 * (c2 + 1)],
                                     start=True, stop=True)
                    nc.scalar.activation(om_q[:, 512 * c2:512 * (c2 + 1)],
                                         pu[:], AF.Sigmoid,
                                         bias=sc(S_BOUT, G), scale=1.0)

                # ---- gate: ct = om*(s2*mm+s0); linear terms s1*mm + s3
                # folded into the broadcast matmul (bcast_mm lhsT) ----
                bt = dpool.tile([G, QF], BF, tag="d")
                nc.vector.tensor_scalar(bt[:], mm_q[0:G, :], sc(S_G2, G),
                                        sc(S_G0, G), ALU.mult, ALU.add)
                ct = qcpool.tile([G, QF], BF, tag="ctq")
                nc.vector.tensor_tensor(ct[:], om_q[:], bt[:], ALU.mult)

                # ---- broadcast gate + final multiply ----
                # cen is re-read from DRAM into transient tiles here so the
                # cg tiles release right after the x-conv, letting the next
                # half's cen loads and x-conv overlap this half's tail.
                for g in range(G):
                    cf = cfpool.tile([C, GR, W], BF, tag="cf")
                    nc.scalar.dma_start(
                        out=cf[:],
                        in_=cen_b[:, base + 3 + 8 * g: base + 3 + 8 * g + 8, :])
                    out_t = outpool.tile([C, GR, W], BF, tag="ot")
                    for h2 in range(2):
                        pg = pbpool.tile([C, 1024], F32, tag="pb")
                        for c2 in range(2):
                            lo = 1024 * h2 + 512 * c2
                            cc = slice(512 * c2, 512 * (c2 + 1))
                            nc.tensor.matmul(
                                pg[:, cc],
                                cbf_sb[0:8, CB_BCT + g * C:CB_BCT + (g + 1) * C],
                                ct[:, lo:lo + 512],
                                start=True, stop=False)
                            nc.tensor.matmul(
                                pg[:, cc],
                                cbf_sb[0:9, CB_BMM + g * C:CB_BMM + (g + 1) * C],
                                mm_q[:, lo:lo + 512],
                                start=False, stop=True)
                        cen_fl = cf[:, 4 * h2:4 * h2 + 4, :].rearrange(
                            "p a b -> p (a b)")
                        out_fl = out_t[:, 4 * h2:4 * h2 + 4, :].rearrange(
                            "p a b -> p (a b)")
                        gb = gbpool.tile([C, 1024], BF, tag="gb")
                        nc.scalar.copy(gb[:], pg[:])
                        fm_eng = (nc.gpsimd if (R_FM_POOL and g % 2 == 1)
                                  else nc.vector)
                        fm_eng.tensor_tensor(out_fl, cen_fl,
                                             gb[:], ALU.mult)
                    # out writes ride the Pool SWDGE ring: keeps both HWDGE
                    # rings (SP=prologue loads, ACT=cf loads) free of tail
                    # writes and costs only idle Q7 descriptor-gen time
                    nc.gpsimd.dma_start(
                        out=out_p[:, base + 8 * g:base + 8 * g + 8, :],
                        in_=out_t[:])

            rep_ctx = (tc.For_i(0, loop_reps, 1) if loop_reps
                       else contextlib.nullcontext())
            with rep_ctx:
                for mh in range(MH):
                    emit_half(mh)
    nc.compile()
    return nc


def _softmax(v):
    e = np.exp(v - v.max())
    return e / e.sum()


def _prep_consts(inp):
    w = _softmax(inp['scale1'])
    v = _softmax(inp['scale2'])
    s3 = _softmax(inp['scale3'])
    inv = inp['bn_gamma'] / np.sqrt(inp['bn_var'] + BN_EPS)
    bnb = inp['bn_beta'] - inp['bn_mean'] * inv

    w_in_blk = np.zeros((C, G, C), np.float32)
    for g in range(G):
        w_in_blk[:, g, CR * g:CR * (g + 1)] = inp['w_in'].T
    b_in_t = np.tile(inp['b_in'], G)[:, None].astype(np.float32)

    bc_blk = np.kron(np.eye(G), inp['bc_w'].T)
    a_w = v[0] + v[1] / 2.0
    b_w = v[2] + v[1] / 2.0
    bc2 = np.stack([a_w * bc_blk, b_w * bc_blk], 1)  # [C, 2, C]

    wout_lhsT = np.kron(np.eye(G), inp['w_out'][0][:, None]).astype(np.float32)
    k_flat = inp['mas_w1'][0, 0].reshape(9)
    mas_lhsT = np.kron(k_flat[:, None], np.eye(G)).astype(np.float32)

    bcast_ct = np.zeros((8, G, C), np.float32)
    for g in range(G):
        bcast_ct[g, g, :] = 1.0
    bcast_mm = np.zeros((9, G, C), np.float32)
    for g in range(G):
        bcast_mm[g, g, :] = s3[1]
    bcast_mm[8, :, :] = s3[3]

    scal = np.zeros((C, 13), np.float32)
    vals = [w[0], w[1] / 4.0, w[2], v[0], v[1] / 2.0, v[2],
            s3[0], s3[1], s3[2], inp['b_out'][0], inp['mas_b1'][0],
            inp['mas_w2'][0, 0], inp['mas_b2'][0]]
    scal[:] = np.asarray(vals, np.float32)[None, :]

    cbf = np.zeros((C, CB_W), np.float32)
    cbf[:, CB_WIN:CB_WIN + G * C] = w_in_blk.reshape(C, G * C)
    cbf[:, CB_BC:CB_BC + 2 * C] = bc2.reshape(C, 2 * C)
    cbf[:, CB_WOUT:CB_WOUT + 8] = wout_lhsT
    cbf[0:72, CB_MAS:CB_MAS + 8] = mas_lhsT
    cbf[0:8, CB_BCT:CB_BCT + G * C] = bcast_ct.reshape(8, G * C)
    cbf[0:9, CB_BMM:CB_BMM + G * C] = bcast_mm.reshape(9, G * C)
    cf32 = np.zeros((C, 18), np.float32)
    cf32[:, 0:1] = b_in_t
    cf32[:, 3:4] = np.tile(inv, G)[:, None].astype(np.float32)
    cf32[:, 4:5] = np.tile(bnb, G)[:, None].astype(np.float32)
    cf32[:, 5:18] = scal
    return {'cbf': cbf.astype(ml_dtypes.bfloat16), 'cf32': cf32,
            'b_in_t': b_in_t}


def _core_inputs(inp, consts, core):
    b, hf = core // 2, core % 2
    r0 = CH * hf
    cen_pad = np.pad(inp['cen'][b], ((0, 0), (3, 3), (0, 0)))
    mas_pad = np.pad(inp['mas'][b, 0], ((1, 9), (1, 1)))
    cen_core = np.ascontiguousarray(cen_pad[:, r0:r0 + CH + 6, :])
    cf32 = consts['cf32'].copy()
    if hf != 0:
        cf32[:, 1:2] = consts['b_in_t']
    if hf != 1:
        cf32[:, 2:3] = consts['b_in_t']
    return {
        'cen_bf': cen_core.astype(ml_dtypes.bfloat16),
        'mas': np.ascontiguousarray(
            mas_pad[r0:r0 + CH + 8, :]).astype(ml_dtypes.bfloat16),
        'cbf': consts['cbf'],
        'cf32': cf32,
    }


def run(inputs, trace=False):
    inp = {k: np.asarray(v) for k, v in inputs.items()}
    consts = _prep_consts(inp)

    if 'nc' not in _CACHE:
        _CACHE['nc'] = build_nc()
    nc = _CACHE['nc']

    in_maps = [_core_inputs(inp, consts, core) for core in range(N_CORES)]
    res = run_bass_kernel_spmd(nc, in_maps, list(range(N_CORES)), trace=trace)

    out = np.empty((4, C, H, W), np.float32)
    for core in range(N_CORES):
        b, hf = core // 2, core % 2
        out[b, :, CH * hf:CH * (hf + 1), :] = res.results[core]['out'].astype(
            np.float32)
    return out, res


def kernel(**inputs):
    return run(inputs)[0]


def bench(inputs, iters=30, reps=0):
    """Time repeated executions with device-resident inputs (no donation).

    Returns (out, per_call_seconds_list). The kernel writes every output
    element, so dropping the zero-buffer donation is safe.
    """
    import time
    import jax
    from jax.sharding import Mesh, PartitionSpec
    from jax.experimental.shard_map import shard_map
    from concourse import bass2jax

    inp = {k: np.asarray(v) for k, v in inputs.items()}
    consts = _prep_consts(inp)
    key = ('nc', reps)
    if key not in _CACHE:
        _CACHE[key] = build_nc(loop_reps=reps)
    nc = _CACHE[key]

    in_maps = [_core_inputs(inp, consts, core) for core in range(N_CORES)]

    bass2jax.install_neuronx_cc_hook()
    in_names, out_names, out_avals, zero_outs = [], [], [], []
    for alloc in nc.m.functions[0].allocations:
        if not isinstance(mybir.MemoryLocationSet, type) or not isinstance(
                alloc, mybir.MemoryLocationSet):
            continue
        name = alloc.memorylocations[0].name
        pname = (nc.partition_id_tensor.name if nc.partition_id_tensor
                 else None)
        if alloc.kind == "ExternalInput":
            if name != pname:
                in_names.append(name)
        elif alloc.kind == "ExternalOutput":
            out_names.append(name)
            out_avals.append(jax.core.ShapedArray(
                tuple(alloc.tensor_shape), mybir.dt.np(alloc.dtype)))
            zero_outs.append(np.zeros(tuple(alloc.tensor_shape),
                                      mybir.dt.np(alloc.dtype)))
    n_params = len(in_names)
    all_names = in_names + out_names
    if nc.partition_id_tensor:
        all_names = all_names + [nc.partition_id_tensor.name]

    def _body(*args):
        operands = list(args)
        if nc.partition_id_tensor:
            operands.append(bass2jax.partition_id_tensor())
        outs = bass2jax._bass_exec_p.bind(
            *operands,
            out_avals=tuple(out_avals),
            in_names=tuple(all_names),
            out_names=tuple(out_names),
            lowering_input_output_aliases=(),
            sim_require_finite=True,
            sim_require_nnan=True,
            nc=nc,
        )
        return tuple(outs)

    devices = jax.devices()[:N_CORES]
    mesh = Mesh(np.asarray(devices), ("core",))
    nin = n_params + len(out_names)
    sharded = jax.jit(
        shard_map(_body, mesh=mesh,
                  in_specs=(PartitionSpec("core"),) * nin,
                  out_specs=(PartitionSpec("core"),) * len(out_names),
                  check_rep=False),
        donate_argnums=tuple(range(n_params, n_params + len(out_names))),
        keep_unused=True,
    )
    concat_in = [np.concatenate([in_maps[c][nm] for c in range(N_CORES)], 0)
                 for nm in in_names]
    concat_zero = [np.zeros((N_CORES * z.shape[0], *z.shape[1:]), z.dtype)
                   for z in zero_outs]
    sh = jax.sharding.NamedSharding(mesh, PartitionSpec("core"))
    dev_in = [jax.device_put(a, sh) for a in concat_in]
    prev = jax.device_put(concat_zero[0], sh)

    outs = sharded(*dev_in, prev)
    jax.block_until_ready(outs)
    result = np.asarray(outs[0]).copy()
    prev = outs[0]
    times = []
    for _ in range(iters):
        t0 = time.perf_counter()
        outs = sharded(*dev_in, prev)
        jax.block_until_ready(outs)
        times.append(time.perf_counter() - t0)
        prev = outs[0]

    full = np.empty((4, C, H, W), np.float32)
    arr = result.reshape(N_CORES, C, CH, W)
    for core in range(N_CORES):
        b, hf = core // 2, core % 2
        full[b, :, CH * hf:CH * (hf + 1), :] = arr[core].astype(np.float32)
    return full, times
